# revision 57
# baseline (speedup 1.0000x reference)
"""Causal self-attention for B=4, L=2048, D=768, H=6 on 8 TRN2 NeuronCores.

Sharding: 8 cores = 4 batches x 2 head-groups (3 heads / 384 hidden each).
All device math bf16 (fp32 PSUM accumulation), single merged pipeline:
the QKV projection of q-group g+1 runs as PE filler inside the attention
stream of q-group g, so the ACT exp latency/throughput never exposes PE
idle. x^T is pre-transposed on the host. Softmax denominators via DVE
bf16 accumulation (2x perf mode) + GPSIMD partition_all_reduce (no PE
pass, no PSUM bank); diagonal-block triangle masks on DVE; projection
staged PSUM->SBUF (DVE/ACT alternating, bf16) then DMA'd; the last
group's projection is split by head so its h0/h1 partials overlap the
final softmax-normalization chain. Dummy warm-up matmuls bridge the
startup DMA latency so the PE p-state ramp is warm when real work lands.
Host sums the two head-group partials per batch and adds bv@Wo + bo
(softmax rows sum to 1, so the bv term commutes out exactly).
"""

import math

import numpy as np
import ml_dtypes

import concourse.bacc as bacc
import concourse.bass_isa as bass_isa
import concourse.mybir as mybir
import concourse.tile as tile
from concourse.bass_utils import run_bass_kernel_spmd

F32 = mybir.dt.float32
BF16 = mybir.dt.bfloat16
EXP = mybir.ActivationFunctionType.Exp
IDENT = mybir.ActivationFunctionType.Identity

B = 4
L = 2048
D = 768
HEADS = 6
HD = 128
HPC = 3          # heads per core
HG = HPC * HD    # 384: per-core slice of the hidden dim
CB = D // 128    # 6 contraction chunks
SCALE = 1.0 / math.sqrt(HD)
N_CORES = 8


def build_nc(L_=L):
    NQG = L_ // 512   # 512-wide q groups

    nc = bacc.Bacc("TRN2", target_bir_lowering=False, debug=False)
    xT_d = nc.dram_tensor("xT", [D, L_], BF16, kind="ExternalInput").ap()
    wq_d = nc.dram_tensor("wq", [D, HG], BF16, kind="ExternalInput").ap()
    wk_d = nc.dram_tensor("wk", [D, HG], BF16, kind="ExternalInput").ap()
    wv_d = nc.dram_tensor("wv", [D, HG], BF16, kind="ExternalInput").ap()
    wo_d = nc.dram_tensor("wo", [HG, D], BF16, kind="ExternalInput").ap()
    bq_d = nc.dram_tensor("bq", [HG], F32, kind="ExternalInput").ap()
    bk_d = nc.dram_tensor("bk", [HG], F32, kind="ExternalInput").ap()
    tri_d = nc.dram_tensor("tri", [128, 128], BF16, kind="ExternalInput").ap()
    y_d = nc.dram_tensor("y", [L_, D], BF16, kind="ExternalOutput").ap()

    xT_r = xT_d.rearrange("(c p) l -> p c l", p=128)

    with tile.TileContext(nc) as tc:
        with (
            tc.tile_pool(name="persist", bufs=1) as pp,
            tc.tile_pool(name="qkv_sb", bufs=1) as pqkv,
            tc.tile_pool(name="xT", bufs=4) as pxt,
            tc.tile_pool(name="at_pool", bufs=12) as pat,
            tc.tile_pool(name="acc_pool", bufs=4) as pacc,
            tc.tile_pool(name="z_pool", bufs=3) as pz,
            tc.tile_pool(name="nrm_sb", bufs=3) as pn,
            tc.tile_pool(name="ysb_pool", bufs=4) as pysb,
            tc.tile_pool(name="ps_g", bufs=2, space="PSUM") as ps_g,
            tc.tile_pool(name="ps_s", bufs=2, space="PSUM") as ps_s,
            tc.tile_pool(name="ps_o", bufs=2, space="PSUM") as ps_o,
        ):
            # warmup scrap first: Pool boots earliest and must memset this
            # before its SWDGE descriptor-generation work queues up
            scrap = pp.tile([128, 128], BF16)
            nc.gpsimd.memset(scrap, 0.0)
            # small constants on the SWDGE (gpsimd) queue
            bq_sb = pp.tile([128, HPC], F32)
            bk_sb = pp.tile([128, HPC], F32)
            nc.gpsimd.dma_start(bq_sb, bq_d.rearrange("(h p) -> p h", p=128))
            nc.gpsimd.dma_start(bk_sb, bk_d.rearrange("(h p) -> p h", p=128))
            # dummy exp: pulls the ACT Exp-table load into the startup shadow
            warm = pp.tile([1, 1], F32)
            nc.scalar.activation(warm, bq_sb[:1, :1], EXP)

            # per-head tensors as separate tiles: keeps the scheduler's
            # dependency tracking precise across heads
            q_ts = [pqkv.tile([128, L_], BF16, name=f"q_t{h}") for h in range(HPC)]
            k_ts = [pqkv.tile([128, L_], BF16, name=f"k_t{h}") for h in range(HPC)]
            v_t = pqkv.tile([128, L_ // 128, HG], BF16)
            o_ts = [pqkv.tile([128, L_], BF16, name=f"o_t{h}") for h in range(HPC)]

            wq_sb = pp.tile([128, CB, HG], BF16)
            wk_sb = pp.tile([128, CB, HG], BF16)
            wv_sb = pp.tile([128, CB, HG], BF16)
            wo_sb = pp.tile([128, HPC, D], BF16)
            tri = pp.tile([128, 128], BF16)

            # ---- startup DMAs: wq/xt0 in halves, rest whole; wo deferred ----
            xts = []
            xt0 = pxt.tile([128, CB, 512], BF16, tag="xt", name="xt")
            wq_r = wq_d.rearrange("(c p) d -> p c d", p=128)
            for half in range(2):
                cs = slice(3 * half, 3 * half + 3)
                nc.scalar.dma_start(wq_sb[:, cs, :], wq_r[:, cs, :])
                nc.sync.dma_start(xt0[:, cs, :], xT_r[:, cs, 0:512])
            xts.append(xt0)

            def issue_xt(g):
                xt = pxt.tile([128, CB, 512], BF16, tag="xt", name="xt")
                nc.sync.dma_start(xt, xT_r[:, :, g * 512 : (g + 1) * 512])
                return xt

            nc.scalar.dma_start(wk_sb, wk_d.rearrange("(c p) d -> p c d", p=128))
            xts.append(issue_xt(1))
            nc.scalar.dma_start(wv_sb, wv_d.rearrange("(c p) d -> p c d", p=128))
            xts.append(issue_xt(2))
            nc.gpsimd.dma_start(tri, tri_d)

            # ---- phase-1 unit emitters ----
            def emit_qk_unit(g, h, which):
                w_sb, t_sb, b_sb = (
                    (wq_sb, q_ts[h], bq_sb) if which == "q" else (wk_sb, k_ts[h], bk_sb)
                )
                hsl = slice(h * 128, (h + 1) * 128)
                pq = ps_g.tile([128, 512], F32, tag="gemm", name="pg")
                for c in range(CB):
                    nc.tensor.matmul(
                        pq, w_sb[:, c, hsl], xts[g][:, c, :],
                        start=(c == 0), stop=(c == CB - 1),
                    )
                nc.scalar.activation(
                    t_sb[:, g * 512 : (g + 1) * 512], pq, IDENT,
                    bias=b_sb[:, h : h + 1],
                )

            def emit_v_unit(g, b):
                lb = g * 4 + b
                pv = ps_g.tile([128, 512], F32, tag="gemm", name="pg")
                for c in range(CB):
                    nc.tensor.matmul(
                        pv[:, :HG], xts[g][:, c, b * 128 : (b + 1) * 128],
                        wv_sb[:, c, :],
                        start=(c == 0), stop=(c == CB - 1),
                    )
                nc.vector.tensor_copy(v_t[:, lb, :], pv[:, :HG])

            def p1_units(g):
                units = []
                for h in range(HPC):
                    units.append(lambda g=g, h=h: emit_qk_unit(g, h, "q"))
                    units.append(lambda g=g, h=h: emit_qk_unit(g, h, "k"))
                for b in range(4):
                    units.append(lambda g=g, b=b: emit_v_unit(g, b))
                return units

            # PE warmup: dummy matmuls on a memset tile keep the tensor
            # engine's p-state ramp alive while the startup DMAs trickle in
            # (any PE idle gap resets the ramp to the slow p-state)
            # scrap PSUM target: first slot of the (startup-idle) S ring
            ps_w = ps_s.tile([128, 2, 512], F32, tag="ps", name="warm")

            def pe_fill(n):
                # n counted in 512-col equivalents; emit 128-col dummies for
                # finer-grained bridging of the startup DMA pacing
                for _ in range(4 * n):
                    nc.tensor.matmul(
                        ps_w[:, 0, :128], scrap, scrap,
                        start=True, stop=True, skip_group_check=True,
                    )

            pe_fill(5)

            # ---- group 0 QKV: chunk-major q (2 heads) so matmuls start as
            # soon as the first wq/xT chunks land ----
            pqs = [ps_g.tile([128, 512], F32, tag="gemm", name="pg")
                   for _ in range(2)]
            for c in range(CB):
                if c == 3:
                    # bridge the gap until the second wq/xT halves land
                    pe_fill(2)
                for h in range(2):
                    nc.tensor.matmul(
                        pqs[h], wq_sb[:, c, h * 128 : (h + 1) * 128],
                        xts[0][:, c, :],
                        start=(c == 0), stop=(c == CB - 1),
                        skip_group_check=True,
                    )
                pe_fill(1)
            for h in range(2):
                nc.scalar.activation(
                    q_ts[h][:, 0:512], pqs[h], IDENT, bias=bq_sb[:, h : h + 1]
                )
            del pqs
            pe_fill(2)
            emit_qk_unit(0, 2, "q")
            for h in range(HPC):
                emit_qk_unit(0, h, "k")
            for b in range(4):
                emit_v_unit(0, b)

            # ---- merged attention + QKV(g+1) + projection stream ----
            flat = []
            for g in range(NQG):
                nb = 2 * (g + 1)
                for h in range(HPC):
                    for pos in range(nb):
                        flat.append((g, h, pos, pos == nb - 1, pos == 0))
            state = {}
            pending = []  # (delay_in_batches, closure)
            fillers = {}  # batch index -> list of closures

            # distribute QKV(g+1) units across attention batches of group g
            mstart = {}
            mi = 0
            for g in range(NQG):
                mstart[g] = mi
                mi += 2 * (g + 1) * HPC
            for g in range(NQG - 1):
                units = p1_units(g + 1)
                nbat = 2 * (g + 1) * HPC
                for j, u in enumerate(units):
                    m = mstart[g] + min(nbat - 1, (j * nbat) // len(units))
                    fillers.setdefault(m, []).append(u)
            # wo load once the startup HWDGE burst has drained; xt3 early in
            # group-1's window (slot frees after group-0's V units read xt0)
            fillers.setdefault(mstart[0], []).insert(
                0,
                lambda: nc.sync.dma_start(
                    wo_sb, wo_d.rearrange("(h p) e -> p h e", p=128)
                ),
            )
            fillers.setdefault(mstart[1], []).insert(
                0, lambda: xts.append(issue_xt(3))
            )

            def nbatches(g):
                return 2 * (g + 1)

            def c0_of(g, kb):
                i = kb - 4 * g
                return 128 * i if i > 0 else 0

            def emit_S(m):
                g, h, j, last, first = flat[m]
                ps = ps_s.tile([128, 2, 512], F32, tag="ps")
                for t in range(2):
                    kb = 2 * j + t
                    c0 = 0 if j == 2 * g else c0_of(g, kb)
                    nc.tensor.matmul(
                        ps[:, t, c0:],
                        k_ts[h][:, kb * 128 : (kb + 1) * 128],
                        q_ts[h][:, g * 512 + c0 : (g + 1) * 512],
                        start=True, stop=True,
                    )
                state[m] = ps

            def emit_rest(m):
                g, h, j, last, first = flat[m]
                ps = state.pop(m)
                if first:
                    state[("po", g, h)] = ps_o.tile(
                        [128, 512], F32, tag="po", name="po"
                    )
                    state[("acc", g, h)] = pacc.tile(
                        [128, 512], BF16, tag="acc", name="acc"
                    )
                po = state[("po", g, h)]
                acc = state[("acc", g, h)]
                at = pat.tile([128, 2, 512], BF16, tag="at")
                if j == 2 * g + 1:
                    for t in range(2):
                        c0 = c0_of(g, 2 * j + t)
                        nc.scalar.activation(
                            at[:, t, c0:], ps[:, t, c0:], EXP, scale=SCALE
                        )
                else:
                    nc.scalar.activation(at, ps, EXP, scale=SCALE)
                for t in range(2):
                    kb = 2 * j + t
                    i = kb - 4 * g
                    c0 = c0_of(g, kb)
                    if i >= 0:
                        nc.vector.tensor_mul(
                            at[:, t, c0 : c0 + 128], at[:, t, c0 : c0 + 128], tri
                        )
                    if first and t == 0:
                        nc.vector.tensor_copy(acc, at[:, 0, :])
                    else:
                        nc.vector.tensor_add(acc[:, c0:], acc[:, c0:], at[:, t, c0:])
                    nc.tensor.matmul(
                        po[:, c0:],
                        v_t[:, kb, h * 128 : (h + 1) * 128],
                        at[:, t, c0:],
                        start=(first and t == 0), stop=(last and t == 1),
                    )

            def emit_par(g, h, lo, hi):
                # Pool-side partition reduce only (runs on the otherwise
                # idle gpsimd queue, displacing nothing)
                def run():
                    acc = state[("acc", g, h)]
                    if ("z", g, h) not in state:
                        state[("z", g, h)] = pz.tile(
                            [128, 512], F32, tag="z", name="z"
                        )
                    nc.gpsimd.partition_all_reduce(
                        state[("z", g, h)][:, lo:hi], acc[:, lo:hi], 128,
                        bass_isa.ReduceOp.add,
                    )
                return run

            def emit_norm(g, h, lo, hi, pop):
                # DVE-side reciprocal + normalize for a column range
                def run():
                    po = state[("po", g, h)]
                    z = state[("z", g, h)]
                    if pop:
                        state.pop(("po", g, h))
                        state.pop(("acc", g, h))
                        state.pop(("z", g, h))
                    recip = pn.tile([128, 512], F32, tag="recip")
                    nc.vector.reciprocal(recip[:, lo:hi], z[:, lo:hi])
                    nc.vector.tensor_mul(
                        o_ts[h][:, g * 512 + lo : g * 512 + hi],
                        po[:, lo:hi], recip[:, lo:hi],
                    )
                return run

            def emit_finalize(g, h, lo=0, hi=512, pop=True):
                par = emit_par(g, h, lo, hi)
                norm = emit_norm(g, h, lo, hi, pop)
                def run():
                    par()
                    norm()
                return run

            ysb_live = {}

            def proj_half(g, b, eh):
                def run():
                    lb = g * 4 + b
                    lsl = slice(lb * 128, (lb + 1) * 128)
                    if (g, b) not in ysb_live:
                        ysb_live[(g, b)] = pysb.tile(
                            [128, 2, 384], BF16, tag="ysb", name="ysb"
                        )
                    ysb = ysb_live[(g, b)]
                    pyp = ps_g.tile([128, 512], F32, tag="gemm", name="pg")
                    for h2 in range(HPC):
                        nc.tensor.matmul(
                            pyp[:, :384],
                            o_ts[h2][:, lsl],
                            wo_sb[:, h2, eh * 384 : (eh + 1) * 384],
                            start=(h2 == 0), stop=(h2 == HPC - 1),
                        )
                    # alternate DVE/ACT so back-to-back projection copies
                    # run in parallel
                    if eh == 0:
                        nc.vector.tensor_copy(ysb[:, eh, :], pyp[:, :384])
                    else:
                        nc.scalar.activation(
                            ysb[:, eh, :], pyp[:, :384],
                            mybir.ActivationFunctionType.Copy,
                        )
                        nc.sync.dma_start(
                            y_d[lb * 128 : (lb + 1) * 128, :].rearrange(
                                "p (u e) -> p u e", u=2
                            ),
                            ysb,
                        )
                        del ysb_live[(g, b)]
                return run

            emit_S(0)
            for m in range(len(flat)):
                if m + 1 < len(flat):
                    emit_S(m + 1)
                nxt = []
                for d, fn in pending:
                    if d <= 0:
                        fn()
                    else:
                        nxt.append((d - 1, fn))
                pending = nxt
                for u in fillers.get(m, ()):
                    u()
                emit_rest(m)
                g, h, j, last, first = flat[m]
                lasthead = g == NQG - 1 and h == HPC - 1
                if lasthead and j == nbatches(g) - 2:
                    # columns [0:256) of acc are complete one batch early
                    # (the final diagonal batch only touches cols >= 256):
                    # run their partition-reduce concurrently on gpsimd
                    pending.append((1, emit_par(g, h, 0, 256)))
                if last:
                    if lasthead:
                        pending.append((1, emit_norm(g, h, 0, 256, False)))
                        pending.append((1, emit_finalize(g, h, 256, 512)))
                    else:
                        pending.append((1, emit_finalize(g, h)))
                    if h == HPC - 1 and g < NQG - 1:
                        # spread the projection half-units across the next
                        # group's batches: they are the PE filler that
                        # absorbs the per-batch ACT exp overhead deficit
                        nnext = 2 * (g + 2) * HPC
                        for i, (b, eh) in enumerate(
                            (b, eh) for b in range(4) for eh in range(2)
                        ):
                            pending.append(
                                (2 + (i * (nnext - 4)) // 8, proj_half(g, b, eh))
                            )
            # ---- tail: last group's projection, split by head so the
            # h0/h1 partial matmuls run during the final softmax chain
            # (borrowing the now-idle S-ring PSUM banks); emitted BEFORE
            # the flushed finalize so they sit earlier in the PE queue ----
            gl = NQG - 1
            pre = [(b, eh) for b in range(3) for eh in range(2)] + [(3, 0)]
            tgts = []
            for _ in range(2):
                tile_s = ps_s.tile([128, 2, 512], F32, tag="ps", name="pyA")
                tgts += [tile_s[:, 0, :384], tile_s[:, 1, :384]]
            for _ in range(2):
                tile_g = ps_g.tile([128, 512], F32, tag="gemm", name="pg")
                tgts.append(tile_g[:, :384])
            # 7th half in the po-ring slot freed by the previous finalize
            tile_o = ps_o.tile([128, 512], F32, tag="po", name="po")
            tgts.append(tile_o[:, :384])
            for (b, eh), tg in zip(pre, tgts):
                lsl = slice((4 * gl + b) * 128, (4 * gl + b + 1) * 128)
                for h2 in (0, 1):
                    nc.tensor.matmul(
                        tg, o_ts[h2][:, lsl],
                        wo_sb[:, h2, eh * 384 : (eh + 1) * 384],
                        start=(h2 == 0), stop=False,
                    )

            for d, fn in sorted(pending, key=lambda p: p[0]):
                fn()
            ysb4 = pp.tile([128, 4, 2, 384], BF16)

            def tail_copy(b, eh, tg):
                if eh == 0:
                    nc.vector.tensor_copy(ysb4[:, b, eh, :], tg)
                else:
                    nc.scalar.activation(
                        ysb4[:, b, eh, :], tg,
                        mybir.ActivationFunctionType.Copy,
                    )

            def tail_store(bpair, eng):
                r0 = (4 * gl + 2 * bpair) * 128
                eng.dma_start(
                    y_d[r0 : r0 + 256, :].rearrange(
                        "(b p) (u e) -> p b u e", p=128, u=2
                    ),
                    ysb4[:, 2 * bpair : 2 * bpair + 2],
                )

            for (b, eh), tg in zip(pre, tgts):
                lsl = slice((4 * gl + b) * 128, (4 * gl + b + 1) * 128)
                nc.tensor.matmul(
                    tg, o_ts[2][:, lsl], wo_sb[:, 2, eh * 384 : (eh + 1) * 384],
                    start=False, stop=True,
                )
                tail_copy(b, eh, tg)
                if b == 1 and eh == 1:
                    tail_store(0, nc.sync)
                if b == 2 and eh == 1:
                    r2 = (4 * gl + 2) * 128
                    nc.sync.dma_start(
                        y_d[r2 : r2 + 128, :].rearrange("p (u e) -> p u e", u=2),
                        ysb4[:, 2],
                    )
            # last half: full 3-matmul unit on a recycled gemm slot
            pyl = ps_g.tile([128, 512], F32, tag="gemm", name="pg")
            lsl = slice((4 * gl + 3) * 128, (4 * gl + 4) * 128)
            for h2 in range(HPC):
                nc.tensor.matmul(
                    pyl[:, :384], o_ts[h2][:, lsl],
                    wo_sb[:, h2, 384:768],
                    start=(h2 == 0), stop=(h2 == HPC - 1),
                )
            tail_copy(3, 1, pyl[:, :384])
            r3 = (4 * gl + 3) * 128
            nc.scalar.dma_start(
                y_d[r3 : r3 + 128, :].rearrange("p (u e) -> p u e", u=2),
                ysb4[:, 3],
            )

    nc.compile()
    return nc


_NC_CACHE = {}


def _get_nc(L_=L):
    if L_ not in _NC_CACHE:
        _NC_CACHE[L_] = build_nc(L_)
    return _NC_CACHE[L_]


def run_sharded(inputs, L_=L, trace=False):
    bf16 = ml_dtypes.bfloat16
    x = np.asarray(inputs["x_input"], dtype=np.float32).astype(bf16)
    xT = np.ascontiguousarray(x.transpose(0, 2, 1))  # [B, D, L]
    tri = np.triu(np.ones((128, 128), dtype=np.float32)).astype(bf16)
    wq = np.asarray(inputs["Wq"], np.float32).astype(bf16)
    wk = np.asarray(inputs["Wk"], np.float32).astype(bf16)
    wv = np.asarray(inputs["Wv"], np.float32).astype(bf16)
    wo = np.asarray(inputs["Wo"], np.float32).astype(bf16)
    bq = np.asarray(inputs["bq"], np.float32)
    bk = np.asarray(inputs["bk"], np.float32)
    in_maps = []
    for c in range(N_CORES):
        b, gslice = c // 2, slice((c % 2) * HG, (c % 2) * HG + HG)
        in_maps.append(
            {
                "xT": xT[b],
                "tri": tri,
                "wq": np.ascontiguousarray(wq[:, gslice]),
                "wk": np.ascontiguousarray(wk[:, gslice]),
                "wv": np.ascontiguousarray(wv[:, gslice]),
                "wo": np.ascontiguousarray(wo[gslice, :]),
                "bq": np.ascontiguousarray(bq[gslice]),
                "bk": np.ascontiguousarray(bk[gslice]),
            }
        )
    nc = _get_nc(L_)
    try:
        res = run_bass_kernel_spmd(nc, in_maps, list(range(N_CORES)), trace=trace)
    except Exception:
        res = run_bass_kernel_spmd(nc, in_maps, list(range(N_CORES)), trace=trace)
    return res


def kernel(**inputs) -> np.ndarray:
    res = run_sharded(inputs)
    bias = (
        np.asarray(inputs["bv"], np.float32) @ np.asarray(inputs["Wo"], np.float32)
        + np.asarray(inputs["bo"], np.float32)
    )
    out = np.empty((B, L, D), dtype=np.float32)
    for b in range(B):
        out[b] = (
            np.asarray(res.results[2 * b]["y"], dtype=np.float32)
            + np.asarray(res.results[2 * b + 1]["y"], dtype=np.float32)
            + bias
        )
    return out
